# revision 25
# baseline (speedup 1.0000x reference)
"""Trainium2 Bass kernel for the ANIMA-Apex recurrent cell (8-core data parallel).

Layout: states transposed [D, B], batch 128 per core. One persistent SBUF
tile STK [80, 128] bf16 holds W(2x) at rows 0:16, a constant-ones row at 16,
I at rows 32:48, A(2x) at rows 64:80 (zeros elsewhere). Row homes are chosen
so that:
  - every matmul operand sits at a partition base in {0,32,64} (PE rule),
  - every elementwise chain stays on one fixed 16-row range (DVE/ACT are
    partition-lane-locked: all operands of an op must share partitions),
  - biases ride for free as weight rows against the ones row.
Each stage's matmuls contract the full stacked state (K=80, zero rows padded
in the weights - free on the PE, time is N cycles) and write their 16-row
pre-activation block to the consuming chain's home rows, different outputs
side by side along the PSUM free dim, so one Tanh instruction activates a
whole stage. sig(x) = (tanh(x/2)+1)/2 with the 1/2 folded into weights
host-side; W/A states stored 2x so the (t'+1)*t forms need no extra scaling
ops; branched enters the A_input PSUM via two +-0.5-identity matmuls.
All matmul-path tensors bf16 (1 PE cycle/row); rel err vs f32 ref ~4e-3.
"""

import numpy as np
import ml_dtypes

BF16 = ml_dtypes.bfloat16
B, T, S, D, O = 1024, 2048, 8, 16, 4
NCORES = 8
BC = B // NCORES          # 128 batch rows per core
E = 64                    # timesteps per chunk (loop body unroll)
GROUPS = ((0, 64), (64, 64))  # (col offset, lanes): phi out bases must be 0/64/96
ENC_N = 4                 # timesteps per encoder matmul (N = 4*128 = 512)

# column index of each lhsT block inside the packed [80, 10*16] weight stack
WI, G, Z, R, H, CG, AG, TH, EL, AIN = range(10)


def _f32(a):
    return np.ascontiguousarray(np.asarray(a, dtype=np.float32))


def _bf(a):
    return np.ascontiguousarray(np.asarray(a, dtype=np.float32).astype(BF16))


def _prep_params(params):
    p = {k: _f32(v) for k, v in params.items()}

    def full(wW, wI, wA, b, sig):
        # lhsT [80, cols] against rhs STK rows [W(2x);1;.;I;.;A(2x)]
        cols = np.asarray(b).shape[-1] if np.ndim(b) else D
        L = np.zeros((80, cols), np.float32)
        s = 0.5 if sig else 1.0
        if wW is not None:
            L[0:D] = s * 0.5 * wW
        L[D] = s * np.asarray(b)
        if wI is not None:
            L[2 * D : 3 * D] = s * wI
        if wA is not None:
            L[4 * D : 5 * D] = s * 0.5 * wA
        return L

    cond_bc = np.repeat(p["cond_w"], D, axis=1)  # (32, 16) broadcast trick
    blocks = [None] * 10
    blocks[WI] = full(p["W_from_W"], p["W_from_I"], p["W_from_A"], np.zeros(D), False)
    blocks[G] = full(None, p["W_gate_w"][:D], p["W_gate_w"][D:], p["W_gate_b"], True)
    blocks[Z] = full(p["I_z_w"][:D], p["I_z_w"][D : 2 * D], p["I_z_w"][2 * D :], p["I_z_b"], True)
    blocks[R] = full(p["I_r_w"][:D], p["I_r_w"][D : 2 * D], p["I_r_w"][2 * D :], p["I_r_b"], True)
    blocks[H] = full(p["I_h_w"][:D], None, p["I_h_w"][2 * D :], p["I_h_b"], False)
    blocks[CG] = full(cond_bc[:D], cond_bc[D:], None, np.full(D, p["cond_b"][0]), True)
    blocks[AG] = full(p["A_gate_w"][:D], p["A_gate_w"][D:], None, p["A_gate_b"], True)
    blocks[TH] = full(None, p["then_w"], None, p["then_b"], False)
    blocks[EL] = full(None, p["else_w"], None, p["else_b"], False)
    blocks[AIN] = full(p["A_from_W"], p["A_from_I"], p["A_from_A"], np.zeros(D), False)
    wstk = np.concatenate(blocks, axis=1)  # (80, 160)

    hri = np.zeros((48, D), np.float32)
    hri[2 * D :] = 0.5 * p["I_h_w"][D : 2 * D]   # rI2 = 2*r*I -> halve
    ident = np.eye(D, dtype=np.float32)
    ids = np.zeros((80, 3 * D), np.float32)
    ids[:D, :D] = ident                           # xe inject (rows 0:16)
    ids[4 * D : 5 * D, D : 2 * D] = 0.5 * ident   # +0.5 u inject (rows 64:80)
    ids[4 * D : 5 * D, 2 * D :] = -0.5 * ident    # -0.5 v inject
    phi = np.zeros((80, O), np.float32)
    phi[4 * D : 5 * D] = 0.5 * p["phi_w"]         # A stored 2x
    enc = np.concatenate([p["W_enc_w"], p["W_enc_b"][None, :]], axis=0)  # (9,16)
    phib = np.tile(p["phi_b"][None, None, :], (BC, E, 1)).reshape(BC, E * O).astype(np.float32)

    stk0 = np.zeros((80, BC), np.float32)
    stk0[D] = 1.0
    return {
        "stk0": _bf(stk0),
        "wstk": _bf(wstk),
        "w_hri": _bf(hri),
        "w_ids": _bf(ids),
        "w_phi": _bf(phi),
        "w_enc": _bf(enc),
        "phib": _f32(phib),
    }


_PARAM_SHAPES = {
    "stk0": (80, BC),
    "wstk": (80, 160),
    "w_hri": (48, D),
    "w_ids": (80, 3 * D),
    "w_phi": (80, O),
    "w_enc": (S + 1, D),
    "phib": (BC, E * O),
}


def _build(t_steps=T):
    import concourse.mybir as mybir
    import concourse.tile as tile
    from concourse import bacc
    from concourse.bass import ds

    f32 = mybir.dt.float32
    bf16 = mybir.dt.bfloat16
    TANH = mybir.ActivationFunctionType.Tanh
    ADD = mybir.AluOpType.add
    SUB = mybir.AluOpType.subtract
    MUL = mybir.AluOpType.mult

    nc = bacc.Bacc("TRN2", target_bir_lowering=False, num_devices=NCORES)

    xin = nc.declare_dram_parameter("x", [S + 1, t_steps, BC], bf16, isOutput=False)
    wp = {}
    for k, shp in _PARAM_SHAPES.items():
        dt = f32 if k == "phib" else bf16
        wp[k] = nc.declare_dram_parameter(k, list(shp), dt, isOutput=False)
    out = nc.declare_dram_parameter("out", [BC, t_steps, O], f32, isOutput=True)

    with tile.TileContext(nc) as tc:
        with (
            tc.tile_pool(name="singles", bufs=1) as singles,
            tc.tile_pool(name="xin_p", bufs=2) as xin_p,
            tc.tile_pool(name="xe_p", bufs=2) as xe_p,
            tc.tile_pool(name="out_p", bufs=2) as out_p,
            tc.tile_pool(name="tmp", bufs=3) as tmp,
            tc.tile_pool(name="pST", bufs=2, space="PSUM") as pST,
            tc.tile_pool(name="pENC", bufs=2, space="PSUM") as pENC,
            tc.tile_pool(name="pPHI", bufs=1, space="PSUM") as pPHI,
        ):
            w = {}
            for k, shp in _PARAM_SHAPES.items():
                dt = f32 if k == "phib" else bf16
                w[k] = singles.tile(list(shp), dt, name=f"w_{k}")
                nc.sync.dma_start(out=w[k][:], in_=wp[k][:])
            ws = w["wstk"]

            def blk(i):
                return ws[:, i * D : (i + 1) * D]

            # per-group persistent state [80, BG]: W(2x) 0:16 | ones 16 | I 32:48 | A(2x) 64:80
            stks = []
            for g, (off, sz) in enumerate(GROUPS):
                st = singles.tile([80, sz], bf16, name=f"stk{g}")
                nc.sync.dma_start(out=st[:], in_=wp["stk0"][:, off : off + sz])
                stks.append(st)

            D2, D4 = 2 * D, 4 * D
            with tc.For_i(0, t_steps, E) as it:
                xt = xin_p.tile([S + 1, E, BC], bf16)
                nc.sync.dma_start(out=xt[:], in_=xin[:, ds(it, E), :])
                ot = out_p.tile([BC, E * O], f32)
                xe = xe_p.tile([D, E * BC], bf16)

                for j in range(E // ENC_N):
                    pe = pENC.tile([D, ENC_N * BC], f32)
                    nc.tensor.matmul(
                        pe[:],
                        w["w_enc"][:],
                        xt[:, j * ENC_N : (j + 1) * ENC_N, :].rearrange("s e b -> s (e b)"),
                        start=True, stop=True,
                    )
                    nc.scalar.activation(
                        out=xe[:, j * ENC_N * BC : (j + 1) * ENC_N * BC],
                        in_=pe[:], func=TANH,
                    )

                pp = pPHI.tile([BC, E * O], f32)

                def group_events(g, off, sz, k):
                    # stage closures for one (group, step); tiles alloc at use
                    st = {}

                    def tile(v, rows, cols):
                        st[v] = tmp.tile([rows, cols], bf16, tag=f"{v}{g}",
                                         name=f"{v}{g}")
                        return st[v]

                    def s_wi():
                        ps = st["ps"] = pST.tile([80, 8 * sz], f32, name=f"ps{g}",
                                                 tag=f"ps{g}")
                        nc.tensor.matmul(ps[:D, :sz], blk(WI), stks[g][:],
                                         start=True, stop=False, skip_group_check=True)
                        nc.tensor.matmul(ps[:D, :sz], w["w_ids"][:D, :D],
                                         xe[:, k * BC + off : k * BC + off + sz],
                                         start=False, stop=True, skip_group_check=True)
                        nc.tensor.matmul(ps[:D, sz : 2 * sz], blk(G), stks[g][:],
                                         start=True, stop=True, skip_group_check=True)

                    def s_tt():
                        nc.scalar.activation(out=tile("tt", D, 2 * sz)[:],
                                             in_=st["ps"][:D, : 2 * sz], func=TANH)

                    def s_wn():
                        nc.vector.scalar_tensor_tensor(
                            out=stks[g][:D, :], in0=st["tt"][:, sz:], scalar=1.0,
                            in1=st["tt"][:, :sz], op0=ADD, op1=MUL)

                    def s_zr():
                        ps = st["ps"]
                        nc.tensor.matmul(ps[D2 : 3 * D, :sz], blk(Z), stks[g][:],
                                         start=True, stop=True, skip_group_check=True)
                        nc.tensor.matmul(ps[D2 : 3 * D, sz : 2 * sz], blk(R), stks[g][:],
                                         start=True, stop=True, skip_group_check=True)

                    def s_tz():
                        nc.scalar.activation(out=tile("tz", 48, 2 * sz)[D2:, :],
                                             in_=st["ps"][D2 : 3 * D, : 2 * sz], func=TANH)

                    def s_ri():
                        nc.vector.scalar_tensor_tensor(
                            out=tile("ri", 48, sz)[D2:, :], in0=st["tz"][D2:, sz:],
                            scalar=1.0, in1=stks[g][D2 : 3 * D, :], op0=ADD, op1=MUL)

                    def s_h():
                        ps = st["ps"]
                        nc.tensor.matmul(ps[D2 : 3 * D, 2 * sz : 3 * sz], blk(H),
                                         stks[g][:], start=True, stop=False,
                                         skip_group_check=True)
                        nc.tensor.matmul(ps[D2 : 3 * D, 2 * sz : 3 * sz],
                                         w["w_hri"][D2:, :], st["ri"][D2:, :],
                                         start=False, stop=True, skip_group_check=True)

                    def s_th():
                        nc.scalar.activation(out=tile("th", 48, sz)[D2:, :],
                                             in_=st["ps"][D2 : 3 * D, 2 * sz : 3 * sz],
                                             func=TANH)

                    def s_dd():
                        nc.vector.tensor_sub(tile("dd", 48, sz)[D2:, :],
                                             st["th"][D2:, :], stks[g][D2 : 3 * D, :])

                    def s_ee():
                        nc.vector.scalar_tensor_tensor(
                            out=tile("ee", 48, sz)[D2:, :], in0=st["tz"][D2:, :sz],
                            scalar=1.0, in1=st["dd"][D2:, :], op0=ADD, op1=MUL)

                    def s_in():
                        nc.vector.scalar_tensor_tensor(
                            out=stks[g][D2 : 3 * D, :], in0=st["ee"][D2:, :], scalar=0.5,
                            in1=stks[g][D2 : 3 * D, :], op0=MUL, op1=ADD)

                    def s_ct():
                        ps = st["ps"]
                        for i, cb in enumerate((CG, AG)):
                            nc.tensor.matmul(ps[D4:, i * sz : (i + 1) * sz],
                                             ws[: 3 * D, cb * D : (cb + 1) * D],
                                             stks[g][: 3 * D, :], start=True, stop=True,
                                             skip_group_check=True)
                        for i, cb in enumerate((TH, EL)):
                            nc.tensor.matmul(ps[D4:, (2 + i) * sz : (3 + i) * sz],
                                             ws[D2 : 3 * D, cb * D : (cb + 1) * D],
                                             stks[g][D2 : 3 * D, :], start=True, stop=True,
                                             skip_group_check=True)

                    def s_t4():
                        nc.scalar.activation(out=tile("t4", 80, 4 * sz)[D4:, :],
                                             in_=st["ps"][D4:, : 4 * sz], func=TANH)

                    def s_uv():
                        t4 = st["t4"]
                        nc.vector.scalar_tensor_tensor(
                            out=tile("uu", 80, sz)[D4:, :], in0=t4[D4:, :sz], scalar=1.0,
                            in1=t4[D4:, 2 * sz : 3 * sz], op0=ADD, op1=MUL)
                        nc.vector.scalar_tensor_tensor(
                            out=tile("vv", 80, sz)[D4:, :], in0=t4[D4:, :sz], scalar=1.0,
                            in1=t4[D4:, 3 * sz :], op0=SUB, op1=MUL)

                    def s_ai():
                        ps = st["ps"]
                        nc.tensor.matmul(ps[D4:, 4 * sz : 5 * sz], blk(AIN), stks[g][:],
                                         start=True, stop=False, skip_group_check=True)
                        nc.tensor.matmul(ps[D4:, 4 * sz : 5 * sz],
                                         w["w_ids"][D4:, D : 2 * D], st["uu"][D4:, :],
                                         start=False, stop=False, skip_group_check=True)
                        nc.tensor.matmul(ps[D4:, 4 * sz : 5 * sz],
                                         w["w_ids"][D4:, 2 * D :], st["vv"][D4:, :],
                                         start=False, stop=True, skip_group_check=True)

                    def s_ta():
                        nc.scalar.activation(out=tile("ta", 80, sz)[D4:, :],
                                             in_=st["ps"][D4:, 4 * sz : 5 * sz], func=TANH)

                    def s_an():
                        nc.vector.scalar_tensor_tensor(
                            out=stks[g][D4:, :], in0=st["t4"][D4:, sz : 2 * sz],
                            scalar=1.0, in1=st["ta"][D4:, :], op0=ADD, op1=MUL)

                    def s_phi():
                        nc.tensor.matmul(pp[off : off + sz, k * O : (k + 1) * O],
                                         stks[g][D4:, :], w["w_phi"][D4:, :],
                                         start=True, stop=True, skip_group_check=True,
                                         tile_position=(D4, off))

                    return [s_wi, s_tt, s_wn, s_zr, s_tz, s_ri, s_h, s_th, s_dd,
                            s_ee, s_in, s_ct, s_t4, s_uv, s_ai, s_ta, s_an, s_phi]

                ev0, ev1 = [], []
                for k in range(E):
                    ev0.extend(group_events(0, GROUPS[0][0], GROUPS[0][1], k))
                    ev1.extend(group_events(1, GROUPS[1][0], GROUPS[1][1], k))
                # emit group 1 shifted half a step behind group 0 so in-order
                # engine queues never head-of-line block one group on the other
                SH = 9
                for i in range(len(ev0) + SH):
                    if i < len(ev0):
                        ev0[i]()
                    if i >= SH:
                        ev1[i - SH]()

                nc.vector.tensor_add(ot[:], pp[:], w["phib"][:])
                nc.sync.dma_start(out=out[:, ds(it, E), :],
                                  in_=ot[:].rearrange("b (e o) -> b e o", o=O))

    nc.finalize()
    return nc


def _run(nc, in_maps, trace):
    import concourse.bass_utils as bu
    from concourse.bass_utils import run_bass_kernel_spmd


    if trace:
        import sys as _sys, types as _types

        try:
            import antenv.axon_hooks  # noqa: F401
        except ImportError:
            import trn_agent_boot.trn_boot as _tb

            _hook = _tb._ntff_profile_via_ctypes("/opt/axon/libaxon_pjrt.so")
            _m = _types.ModuleType("antenv.axon_hooks")
            _m.get_axon_ntff_profile_hook = lambda: _hook
            _sys.modules["antenv.axon_hooks"] = _m
    return run_bass_kernel_spmd(nc, in_maps, core_ids=list(range(NCORES)), trace=trace)


def kernel(x, params, _trace=False, _t_steps=T):
    x = np.asarray(x)
    assert x.shape[0] == B and x.shape[2] == S, x.shape
    x = x[:, :_t_steps]
    pk = _prep_params(params)
    nc = _build(_t_steps)

    in_maps = []
    for c in range(NCORES):
        xc = x[c * BC : (c + 1) * BC].transpose(2, 1, 0)  # (S, t, BC)
        xc = np.concatenate([xc, np.ones((1, _t_steps, BC), np.float32)], axis=0)
        m = {"x": np.ascontiguousarray(xc.astype(BF16))}
        m.update(pk)
        in_maps.append(m)

    res = _run(nc, in_maps, _trace)
    kernel._last_results = res
    full = np.empty((B, _t_steps, O), np.float32)
    for c in range(NCORES):
        full[c * BC : (c + 1) * BC] = res.results[c]["out"]
    return full


# revision 26
# speedup vs baseline: 1.0029x; 1.0029x over previous
"""Trainium2 Bass kernel for the ANIMA-Apex recurrent cell (8-core data parallel).

Layout: states transposed [D, B], batch 128 per core. One persistent SBUF
tile STK [80, 128] bf16 holds W(2x) at rows 0:16, a constant-ones row at 16,
I at rows 32:48, A(2x) at rows 64:80 (zeros elsewhere). Row homes are chosen
so that:
  - every matmul operand sits at a partition base in {0,32,64} (PE rule),
  - every elementwise chain stays on one fixed 16-row range (DVE/ACT are
    partition-lane-locked: all operands of an op must share partitions),
  - biases ride for free as weight rows against the ones row.
Each stage's matmuls contract the full stacked state (K=80, zero rows padded
in the weights - free on the PE, time is N cycles) and write their 16-row
pre-activation block to the consuming chain's home rows, different outputs
side by side along the PSUM free dim, so one Tanh instruction activates a
whole stage. sig(x) = (tanh(x/2)+1)/2 with the 1/2 folded into weights
host-side; W/A states stored 2x so the (t'+1)*t forms need no extra scaling
ops; branched enters the A_input PSUM via two +-0.5-identity matmuls.
All matmul-path tensors bf16 (1 PE cycle/row); rel err vs f32 ref ~4e-3.
"""

import numpy as np
import ml_dtypes

BF16 = ml_dtypes.bfloat16
B, T, S, D, O = 1024, 2048, 8, 16, 4
NCORES = 8
BC = B // NCORES          # 128 batch rows per core
E = 64                    # timesteps per chunk (loop body unroll)
GROUPS = ((0, 64), (64, 64))  # (col offset, lanes): phi out bases must be 0/64/96
ENC_N = 4                 # timesteps per encoder matmul (N = 4*128 = 512)

# column index of each lhsT block inside the packed [80, 10*16] weight stack
WI, G, Z, R, H, CG, AG, TH, EL, AIN = range(10)


def _f32(a):
    return np.ascontiguousarray(np.asarray(a, dtype=np.float32))


def _bf(a):
    return np.ascontiguousarray(np.asarray(a, dtype=np.float32).astype(BF16))


def _prep_params(params):
    p = {k: _f32(v) for k, v in params.items()}

    def full(wW, wI, wA, b, sig):
        # lhsT [80, cols] against rhs STK rows [W(2x);1;.;I;.;A(2x)]
        cols = np.asarray(b).shape[-1] if np.ndim(b) else D
        L = np.zeros((80, cols), np.float32)
        s = 0.5 if sig else 1.0
        if wW is not None:
            L[0:D] = s * 0.5 * wW
        L[D] = s * np.asarray(b)
        if wI is not None:
            L[2 * D : 3 * D] = s * wI
        if wA is not None:
            L[4 * D : 5 * D] = s * 0.5 * wA
        return L

    cond_bc = np.repeat(p["cond_w"], D, axis=1)  # (32, 16) broadcast trick
    blocks = [None] * 10
    blocks[WI] = full(p["W_from_W"], p["W_from_I"], p["W_from_A"], np.zeros(D), False)
    blocks[G] = full(None, p["W_gate_w"][:D], p["W_gate_w"][D:], p["W_gate_b"], True)
    blocks[Z] = full(p["I_z_w"][:D], p["I_z_w"][D : 2 * D], p["I_z_w"][2 * D :], p["I_z_b"], True)
    blocks[R] = full(p["I_r_w"][:D], p["I_r_w"][D : 2 * D], p["I_r_w"][2 * D :], p["I_r_b"], True)
    blocks[H] = full(p["I_h_w"][:D], None, p["I_h_w"][2 * D :], p["I_h_b"], False)
    blocks[CG] = full(cond_bc[:D], cond_bc[D:], None, np.full(D, p["cond_b"][0]), True)
    blocks[AG] = full(p["A_gate_w"][:D], p["A_gate_w"][D:], None, p["A_gate_b"], True)
    blocks[TH] = full(None, p["then_w"], None, p["then_b"], False)
    blocks[EL] = full(None, p["else_w"], None, p["else_b"], False)
    blocks[AIN] = full(p["A_from_W"], p["A_from_I"], p["A_from_A"], np.zeros(D), False)
    wstk = np.concatenate(blocks, axis=1)  # (80, 160)

    hri = np.zeros((48, D), np.float32)
    hri[2 * D :] = 0.5 * p["I_h_w"][D : 2 * D]   # rI2 = 2*r*I -> halve
    ident = np.eye(D, dtype=np.float32)
    ids = np.zeros((80, 3 * D), np.float32)
    ids[:D, :D] = ident                           # xe inject (rows 0:16)
    ids[4 * D : 5 * D, D : 2 * D] = 0.5 * ident   # +0.5 u inject (rows 64:80)
    ids[4 * D : 5 * D, 2 * D :] = -0.5 * ident    # -0.5 v inject
    phi = np.zeros((80, O), np.float32)
    phi[4 * D : 5 * D] = 0.5 * p["phi_w"]         # A stored 2x
    enc = np.concatenate([p["W_enc_w"], p["W_enc_b"][None, :]], axis=0)  # (9,16)
    phib = np.tile(p["phi_b"][None, None, :], (BC, E, 1)).reshape(BC, E * O).astype(np.float32)

    stk0 = np.zeros((80, BC), np.float32)
    stk0[D] = 1.0
    return {
        "stk0": _bf(stk0),
        "wstk": _bf(wstk),
        "w_hri": _bf(hri),
        "w_ids": _bf(ids),
        "w_phi": _bf(phi),
        "w_enc": _bf(enc),
        "phib": _f32(phib),
    }


_PARAM_SHAPES = {
    "stk0": (80, BC),
    "wstk": (80, 160),
    "w_hri": (48, D),
    "w_ids": (80, 3 * D),
    "w_phi": (80, O),
    "w_enc": (S + 1, D),
    "phib": (BC, E * O),
}


def _build(t_steps=T):
    import concourse.mybir as mybir
    import concourse.tile as tile
    from concourse import bacc
    from concourse.bass import ds

    f32 = mybir.dt.float32
    bf16 = mybir.dt.bfloat16
    TANH = mybir.ActivationFunctionType.Tanh
    ADD = mybir.AluOpType.add
    SUB = mybir.AluOpType.subtract
    MUL = mybir.AluOpType.mult

    nc = bacc.Bacc("TRN2", target_bir_lowering=False, num_devices=NCORES)

    xin = nc.declare_dram_parameter("x", [S + 1, t_steps, BC], bf16, isOutput=False)
    wp = {}
    for k, shp in _PARAM_SHAPES.items():
        dt = f32 if k == "phib" else bf16
        wp[k] = nc.declare_dram_parameter(k, list(shp), dt, isOutput=False)
    out = nc.declare_dram_parameter("out", [BC, t_steps, O], f32, isOutput=True)

    with tile.TileContext(nc) as tc:
        with (
            tc.tile_pool(name="singles", bufs=1) as singles,
            tc.tile_pool(name="xin_p", bufs=2) as xin_p,
            tc.tile_pool(name="xe_p", bufs=2) as xe_p,
            tc.tile_pool(name="out_p", bufs=2) as out_p,
            tc.tile_pool(name="tmp", bufs=3) as tmp,
            tc.tile_pool(name="pST", bufs=2, space="PSUM") as pST,
            tc.tile_pool(name="pENC", bufs=2, space="PSUM") as pENC,
            tc.tile_pool(name="pPHI", bufs=1, space="PSUM") as pPHI,
        ):
            w = {}
            for k, shp in _PARAM_SHAPES.items():
                dt = f32 if k == "phib" else bf16
                w[k] = singles.tile(list(shp), dt, name=f"w_{k}")
                nc.sync.dma_start(out=w[k][:], in_=wp[k][:])
            ws = w["wstk"]

            def blk(i):
                return ws[:, i * D : (i + 1) * D]

            # per-group persistent state [80, BG]: W(2x) 0:16 | ones 16 | I 32:48 | A(2x) 64:80
            stks = []
            for g, (off, sz) in enumerate(GROUPS):
                st = singles.tile([80, sz], bf16, name=f"stk{g}")
                nc.sync.dma_start(out=st[:], in_=wp["stk0"][:, off : off + sz])
                stks.append(st)

            D2, D4 = 2 * D, 4 * D
            with tc.For_i(0, t_steps, E) as it:
                xt = xin_p.tile([S + 1, E, BC], bf16)
                nc.sync.dma_start(out=xt[:], in_=xin[:, ds(it, E), :])
                ot = out_p.tile([BC, E * O], f32)
                xe = xe_p.tile([D, E * BC], bf16)

                for j in range(E // ENC_N):
                    pe = pENC.tile([D, ENC_N * BC], f32)
                    nc.tensor.matmul(
                        pe[:],
                        w["w_enc"][:],
                        xt[:, j * ENC_N : (j + 1) * ENC_N, :].rearrange("s e b -> s (e b)"),
                        start=True, stop=True,
                    )
                    nc.scalar.activation(
                        out=xe[:, j * ENC_N * BC : (j + 1) * ENC_N * BC],
                        in_=pe[:], func=TANH,
                    )

                pp = pPHI.tile([BC, E * O], f32)
                for k in range(E):
                    # per-group PSUM bank regions (cols in units of sz):
                    # wi [0:16,0:1] | g [0:16,1:2] | z [32:48,0:1] | r [32:48,1:2]
                    # h [32:48,2:3] | cg/ag/T/E [64:80,0:4] | ai [64:80,4:5]
                    ps = [pST.tile([80, 8 * sz], f32, name=f"ps{g}", tag=f"ps{g}")
                          for g, (off, sz) in enumerate(GROUPS)]
                    tl = {v: [None] * len(GROUPS)
                          for v in ("tt", "tz", "th", "dd", "ee", "t4", "uu", "vv", "ta", "ri")}

                    def _t(v, g, rows, cols):
                        tl[v][g] = tmp.tile([rows, cols], bf16, tag=f"{v}{g}", name=f"{v}{g}")
                        return tl[v][g]

                    for g, (off, sz) in enumerate(GROUPS):
                        nc.tensor.matmul(ps[g][:D, :sz], blk(WI), stks[g][:],
                                         start=True, stop=False, skip_group_check=True)
                        nc.tensor.matmul(ps[g][:D, :sz], w["w_ids"][:D, :D],
                                         xe[:, k * BC + off : k * BC + off + sz],
                                         start=False, stop=True, skip_group_check=True)
                    for g, (off, sz) in enumerate(GROUPS):
                        nc.tensor.matmul(ps[g][:D, sz : 2 * sz], blk(G), stks[g][:],
                                         start=True, stop=True, skip_group_check=True)
                    for g, (off, sz) in enumerate(GROUPS):
                        nc.scalar.activation(out=_t("tt", g, D, 2 * sz)[:],
                                             in_=ps[g][:D, : 2 * sz], func=TANH)
                    for g, (off, sz) in enumerate(GROUPS):  # W_new(2x)
                        nc.vector.scalar_tensor_tensor(
                            out=stks[g][:D, :], in0=tl["tt"][g][:, sz:], scalar=1.0,
                            in1=tl["tt"][g][:, :sz], op0=ADD, op1=MUL)
                    for g, (off, sz) in enumerate(GROUPS):
                        nc.tensor.matmul(ps[g][D2 : 3 * D, :sz], blk(Z), stks[g][:],
                                         start=True, stop=True, skip_group_check=True)
                        nc.tensor.matmul(ps[g][D2 : 3 * D, sz : 2 * sz], blk(R), stks[g][:],
                                         start=True, stop=True, skip_group_check=True)
                    for g, (off, sz) in enumerate(GROUPS):
                        nc.scalar.activation(out=_t("tz", g, 48, 2 * sz)[D2:, :],
                                             in_=ps[g][D2 : 3 * D, : 2 * sz], func=TANH)
                    for g, (off, sz) in enumerate(GROUPS):  # ri = (r'+1)*I_old = 2rI
                        nc.vector.scalar_tensor_tensor(
                            out=_t("ri", g, 48, sz)[D2:, :], in0=tl["tz"][g][D2:, sz:], scalar=1.0,
                            in1=stks[g][D2 : 3 * D, :], op0=ADD, op1=MUL)
                    for g, (off, sz) in enumerate(GROUPS):
                        nc.tensor.matmul(ps[g][D2 : 3 * D, 2 * sz : 3 * sz], blk(H),
                                         stks[g][:], start=True, stop=False,
                                         skip_group_check=True)
                        nc.tensor.matmul(ps[g][D2 : 3 * D, 2 * sz : 3 * sz],
                                         w["w_hri"][D2:, :], tl["ri"][g][D2:, :],
                                         start=False, stop=True, skip_group_check=True)
                    for g, (off, sz) in enumerate(GROUPS):
                        nc.scalar.activation(out=_t("th", g, 48, sz)[D2:, :],
                                             in_=ps[g][D2 : 3 * D, 2 * sz : 3 * sz],
                                             func=TANH)
                    for g, (off, sz) in enumerate(GROUPS):  # I_new = I + (z'+1)(h-I)/2
                        nc.vector.tensor_sub(_t("dd", g, 48, sz)[D2:, :], tl["th"][g][D2:, :],
                                             stks[g][D2 : 3 * D, :])
                    for g, (off, sz) in enumerate(GROUPS):
                        nc.vector.scalar_tensor_tensor(
                            out=_t("ee", g, 48, sz)[D2:, :], in0=tl["tz"][g][D2:, :sz], scalar=1.0,
                            in1=tl["dd"][g][D2:, :], op0=ADD, op1=MUL)
                    for g, (off, sz) in enumerate(GROUPS):
                        nc.vector.scalar_tensor_tensor(
                            out=stks[g][D2 : 3 * D, :], in0=tl["ee"][g][D2:, :], scalar=0.5,
                            in1=stks[g][D2 : 3 * D, :], op0=MUL, op1=ADD)
                    for i, cb in enumerate((CG, AG)):     # K=48: rows 0:48 [W;1;I]
                        for g, (off, sz) in enumerate(GROUPS):
                            nc.tensor.matmul(ps[g][D4:, i * sz : (i + 1) * sz],
                                             ws[: 3 * D, cb * D : (cb + 1) * D],
                                             stks[g][: 3 * D, :], start=True, stop=True,
                                             skip_group_check=True)
                    for i, cb in enumerate((TH, EL)):     # K=16: rows 32:48 [I]
                        for g, (off, sz) in enumerate(GROUPS):
                            nc.tensor.matmul(ps[g][D4:, (2 + i) * sz : (3 + i) * sz],
                                             ws[D2 : 3 * D, cb * D : (cb + 1) * D],
                                             stks[g][D2 : 3 * D, :], start=True, stop=True,
                                             skip_group_check=True)
                    for g, (off, sz) in enumerate(GROUPS):
                        nc.scalar.activation(out=_t("t4", g, 80, 4 * sz)[D4:, :],
                                             in_=ps[g][D4:, : 4 * sz], func=TANH)
                    for g, (off, sz) in enumerate(GROUPS):  # uu = 2cgT; vv = -2(1-cg)Eo
                        nc.vector.scalar_tensor_tensor(
                            out=_t("uu", g, 80, sz)[D4:, :], in0=tl["t4"][g][D4:, :sz], scalar=1.0,
                            in1=tl["t4"][g][D4:, 2 * sz : 3 * sz], op0=ADD, op1=MUL)
                        nc.vector.scalar_tensor_tensor(
                            out=_t("vv", g, 80, sz)[D4:, :], in0=tl["t4"][g][D4:, :sz], scalar=1.0,
                            in1=tl["t4"][g][D4:, 3 * sz :], op0=SUB, op1=MUL)
                    for g, (off, sz) in enumerate(GROUPS):
                        nc.tensor.matmul(ps[g][D4:, 4 * sz : 5 * sz], blk(AIN), stks[g][:],
                                         start=True, stop=False, skip_group_check=True)
                        nc.tensor.matmul(ps[g][D4:, 4 * sz : 5 * sz],
                                         w["w_ids"][D4:, D : 2 * D], tl["uu"][g][D4:, :],
                                         start=False, stop=False, skip_group_check=True)
                        nc.tensor.matmul(ps[g][D4:, 4 * sz : 5 * sz],
                                         w["w_ids"][D4:, 2 * D :], tl["vv"][g][D4:, :],
                                         start=False, stop=True, skip_group_check=True)
                    for g, (off, sz) in enumerate(GROUPS):
                        nc.scalar.activation(out=_t("ta", g, 80, sz)[D4:, :],
                                             in_=ps[g][D4:, 4 * sz : 5 * sz], func=TANH)
                    for g, (off, sz) in enumerate(GROUPS):  # A_new(2x)
                        nc.vector.scalar_tensor_tensor(
                            out=stks[g][D4:, :], in0=tl["t4"][g][D4:, sz : 2 * sz],
                            scalar=1.0, in1=tl["ta"][g][D4:, :], op0=ADD, op1=MUL)
                    for g, (off, sz) in enumerate(GROUPS):
                        nc.tensor.matmul(pp[off : off + sz, k * O : (k + 1) * O],
                                         stks[g][D4:, :], w["w_phi"][D4:, :],
                                         start=True, stop=True, skip_group_check=True,
                                         tile_position=(D4, off))

                nc.vector.tensor_add(ot[:], pp[:], w["phib"][:])
                nc.sync.dma_start(out=out[:, ds(it, E), :],
                                  in_=ot[:].rearrange("b (e o) -> b e o", o=O))

    nc.finalize()
    return nc


def _run(nc, in_maps, trace):
    import concourse.bass_utils as bu
    from concourse.bass_utils import run_bass_kernel_spmd


    if trace:
        import sys as _sys, types as _types

        try:
            import antenv.axon_hooks  # noqa: F401
        except ImportError:
            import trn_agent_boot.trn_boot as _tb

            _hook = _tb._ntff_profile_via_ctypes("/opt/axon/libaxon_pjrt.so")
            _m = _types.ModuleType("antenv.axon_hooks")
            _m.get_axon_ntff_profile_hook = lambda: _hook
            _sys.modules["antenv.axon_hooks"] = _m
    return run_bass_kernel_spmd(nc, in_maps, core_ids=list(range(NCORES)), trace=trace)


def kernel(x, params, _trace=False, _t_steps=T):
    x = np.asarray(x)
    assert x.shape[0] == B and x.shape[2] == S, x.shape
    x = x[:, :_t_steps]
    pk = _prep_params(params)
    nc = _build(_t_steps)

    in_maps = []
    for c in range(NCORES):
        xc = x[c * BC : (c + 1) * BC].transpose(2, 1, 0)  # (S, t, BC)
        xc = np.concatenate([xc, np.ones((1, _t_steps, BC), np.float32)], axis=0)
        m = {"x": np.ascontiguousarray(xc.astype(BF16))}
        m.update(pk)
        in_maps.append(m)

    res = _run(nc, in_maps, _trace)
    kernel._last_results = res
    full = np.empty((B, _t_steps, O), np.float32)
    for c in range(NCORES):
        full[c * BC : (c + 1) * BC] = res.results[c]["out"]
    return full
